# revision 1
# baseline (speedup 1.0000x reference)
"""Trainium2 Bass kernel: 8-NN retrieval with inverse-distance weighting.

Problem (full): data1 [4096, 1024] queries, data2 [8192, 1024] database.
  dist = pairwise Euclidean distances; top-8 nearest per query;
  w = 1/(dist+0.1); out = weighted average of the 8 neighbor vectors.

Sharding: data1 row-sharded across 8 NeuronCores (512 queries/core);
data2 replicated. Each core computes
  score[n, m] = q_n . y_m - 0.5*||y_m||^2   (argmax score == argmin dist)
via fp32 TensorE matmuls (queries/database pre-transposed on host so the
contraction dim lands on SBUF partitions), selects top-8 per row with the
DVE Max8/MaxIndex instructions, reconstructs dist_k = sqrt(x2_n - 2*score_k),
gathers the 8 neighbor rows by indirect DMA, and DVE-accumulates the
weighted average.
"""

import sys

sys.path.insert(0, "/opt/trn_rl_repo")

import numpy as np

P = 128
D = 1024
M = 8192
NQ = 512          # queries per core
KD = D // P       # 8 contraction tiles
MC = 512          # m-chunk width (one PSUM bank)
NMC = M // MC     # 16
NT = NQ // P      # 4 query tiles per core
K = 8
CONST = 0.1
EPS = 1e-12
N_CORES = 8

_CACHE = {}


def _build_nc(finalize=True):
    import concourse.bacc as bacc
    import concourse.bass as bass
    import concourse.mybir as mybir
    from concourse.masks import make_identity
    from concourse.tile import TileContext

    f32 = mybir.dt.float32
    u32 = mybir.dt.uint32
    AF = mybir.ActivationFunctionType
    OP = mybir.AluOpType

    n_tile_groups = ((0, 2), (2, 4))

    nc = bacc.Bacc()

    qt = nc.dram_tensor("qt", [D, NQ], f32, kind="ExternalInput")
    qn = nc.dram_tensor("qn", [NQ, D], f32, kind="ExternalInput")
    dbt = nc.dram_tensor("dbt", [D, M], f32, kind="ExternalInput")
    dbn = nc.dram_tensor("dbn", [M, D], f32, kind="ExternalInput")
    out = nc.dram_tensor("out", [NQ, D], f32, kind="ExternalOutput")

    with TileContext(nc) as tc:
        with (
            tc.tile_pool(name="persist", bufs=1) as pp,
            tc.tile_pool(name="stream", bufs=2) as sp,
            tc.tile_pool(name="scorep", bufs=1) as scp,
            tc.tile_pool(name="neighp", bufs=1) as nbp,
            tc.tile_pool(name="psum", bufs=4, space="PSUM") as psp,
            tc.tile_pool(name="psum1", bufs=1, space="PSUM") as ps1,
        ):
            # ---- identity (for the y2 transpose) ----
            ident = pp.tile([P, P], f32)
            make_identity(nc, ident[:])

            # ---- queries, d-major (matmul lhsT), columns kd*NQ + n ----
            qt_sb = pp.tile([P, KD * NQ], f32)
            nc.sync.dma_start(
                out=qt_sb[:].rearrange("p (kd n) -> p kd n", kd=KD),
                in_=qt[:, :].rearrange("(kd p) n -> p kd n", p=P),
            )

            # ---- x2[n] = ||q_n||^2, one column per n-tile ----
            x2col = pp.tile([P, NT], f32)
            junk = pp.tile([P, D], f32)  # ACT mandatory full output, discarded
            for t in range(NT):
                qn_t = sp.tile([P, D], f32, tag="qn_t", name="qn_t")
                nc.sync.dma_start(out=qn_t[:], in_=qn[t * P : (t + 1) * P, :])
                nc.scalar.activation(
                    out=junk[:],
                    in_=qn_t[:],
                    func=AF.Square,
                    accum_out=x2col[:, t : t + 1],
                )

            # ---- y2 pass: y2blk[p, t] = ||y_{t*128+p}||^2 ----
            MT = M // P  # 64
            y2blk = pp.tile([P, MT], f32)
            for t in range(MT):
                dbn_t = sp.tile([P, D], f32, tag="dbn_t", name="dbn_t")
                nc.sync.dma_start(out=dbn_t[:], in_=dbn[t * P : (t + 1) * P, :])
                nc.scalar.activation(
                    out=junk[:],
                    in_=dbn_t[:],
                    func=AF.Square,
                    accum_out=y2blk[:, t : t + 1],
                )
            # transpose then scale by -0.5: y2t[t, p] = -0.5*y2[t*128+p]
            y2t_ps = ps1.tile([MT, P], f32)
            nc.tensor.transpose(out=y2t_ps[:], in_=y2blk[:], identity=ident[:])
            y2t = pp.tile([MT, P], f32)
            nc.vector.tensor_scalar_mul(y2t[:], y2t_ps[:], -0.5)
            # flatten into partition row 0 of y2rep: y2rep[0, t*128+j] = y2t[t, j]
            y2rep = pp.tile([P, M], f32)
            nc.sync.dma_start(
                out=y2rep[0:1, :].rearrange("o (t j) -> o t j", t=MT),
                in_=y2t[:, :],
            )
            # doubling broadcast down the partitions
            s = 1
            while s < P:
                nc.sync.dma_start(out=y2rep[s : 2 * s, :], in_=y2rep[0:s, :])
                s *= 2

            # ---- selection scratch ----
            top8 = pp.tile([P, NT * K], f32)
            idx8 = pp.tile([P, NT * K], u32)
            wts = pp.tile([P, NT * K], f32)
            dsc = pp.tile([P, NT * K], f32)  # d2 -> dist -> dist+c scratch
            wsum = pp.tile([P, NT], f32)
            winv = pp.tile([P, NT], f32)

            scores = [
                scp.tile([P, M], f32, tag=f"score{i}", name=f"score{i}")
                for i in range(max(t1 - t0 for t0, t1 in n_tile_groups))
            ]

            for (t0, t1) in n_tile_groups:
                # ---- score matmuls over all m-chunks ----
                for mc in range(NMC):
                    dbt_c = sp.tile([P, KD * MC], f32, tag="dbt_c", name="dbt_c")
                    nc.sync.dma_start(
                        out=dbt_c[:].rearrange("p (kd j) -> p kd j", kd=KD),
                        in_=dbt[:, mc * MC : (mc + 1) * MC].rearrange(
                            "(kd p) j -> p kd j", p=P
                        ),
                    )
                    for t in range(t0, t1):
                        ps = psp.tile([P, MC], f32, tag="mmps", name="mmps")
                        for d in range(KD):
                            nc.tensor.matmul(
                                ps[:],
                                lhsT=qt_sb[:, d * NQ + t * P : d * NQ + t * P + P],
                                rhs=dbt_c[:, d * MC : (d + 1) * MC],
                                start=(d == 0),
                                stop=(d == KD - 1),
                            )
                        # evict: score = s + (-0.5*y2)
                        nc.vector.tensor_tensor(
                            out=scores[t - t0][:, mc * MC : (mc + 1) * MC],
                            in0=ps[:],
                            in1=y2rep[:, mc * MC : (mc + 1) * MC],
                            op=OP.add,
                        )

                # ---- top-8 selection + weights + gather + weighted average ----
                for t in range(t0, t1):
                    sc = scores[t - t0]
                    sl = slice(t * K, (t + 1) * K)
                    nc.vector.max(out=top8[:, sl], in_=sc[:])
                    nc.vector.max_index(
                        out=idx8[:, sl], in_max=top8[:, sl], in_values=sc[:]
                    )
                    # d2 = x2 - 2*score  (>= 0 up to rounding)
                    nc.vector.scalar_tensor_tensor(
                        out=dsc[:, sl],
                        in0=top8[:, sl],
                        scalar=-2.0,
                        in1=x2col[:, t : t + 1].to_broadcast([P, K]),
                        op0=OP.mult,
                        op1=OP.add,
                    )
                    nc.vector.tensor_scalar_max(dsc[:, sl], dsc[:, sl], EPS)
                    nc.scalar.activation(out=dsc[:, sl], in_=dsc[:, sl], func=AF.Sqrt)
                    nc.vector.tensor_scalar_add(dsc[:, sl], dsc[:, sl], CONST)
                    nc.vector.reciprocal(out=wts[:, sl], in_=dsc[:, sl])
                    nc.vector.reduce_sum(
                        out=wsum[:, t : t + 1],
                        in_=wts[:, sl],
                        axis=mybir.AxisListType.X,
                    )
                    nc.vector.reciprocal(out=winv[:, t : t + 1], in_=wsum[:, t : t + 1])

                    acc = sp.tile([P, D], f32, tag="acc", name="acc")
                    for k in range(K):
                        nb = nbp.tile([P, D], f32, tag=f"nb{k}", name=f"nb{k}")
                        nc.gpsimd.indirect_dma_start(
                            out=nb[:],
                            out_offset=None,
                            in_=dbn[:, :],
                            in_offset=bass.IndirectOffsetOnAxis(
                                ap=idx8[:, t * K + k : t * K + k + 1], axis=0
                            ),
                        )
                        if k == 0:
                            nc.vector.tensor_scalar_mul(
                                acc[:], nb[:], wts[:, t * K : t * K + 1]
                            )
                        else:
                            nc.vector.scalar_tensor_tensor(
                                out=acc[:],
                                in0=nb[:],
                                scalar=wts[:, t * K + k : t * K + k + 1],
                                in1=acc[:],
                                op0=OP.mult,
                                op1=OP.add,
                            )
                    nc.vector.tensor_scalar_mul(acc[:], acc[:], winv[:, t : t + 1])
                    nc.sync.dma_start(out=out[t * P : (t + 1) * P, :], in_=acc[:])

    if finalize:
        nc.finalize()
    return nc


def _shard_inputs(data1, data2):
    data1 = np.ascontiguousarray(np.asarray(data1, dtype=np.float32))
    data2 = np.ascontiguousarray(np.asarray(data2, dtype=np.float32))
    dbt = np.ascontiguousarray(data2.T)
    in_maps = []
    for c in range(N_CORES):
        q = data1[c * NQ : (c + 1) * NQ]
        in_maps.append(
            {
                "qt": np.ascontiguousarray(q.T),
                "qn": np.ascontiguousarray(q),
                "dbt": dbt,
                "dbn": data2,
            }
        )
    return in_maps


def _run(data1, data2, trace=False, trace_kwargs=None):
    from concourse.bass_utils import run_bass_kernel_spmd

    nc = _CACHE.get("nc")
    if nc is None:
        nc = _build_nc()
        _CACHE["nc"] = nc
    in_maps = _shard_inputs(data1, data2)
    res = run_bass_kernel_spmd(
        nc,
        in_maps,
        core_ids=list(range(N_CORES)),
        trace=trace,
        trace_kwargs=trace_kwargs or {},
    )
    full = np.concatenate([res.results[c]["out"] for c in range(N_CORES)], axis=0)
    return full, res


def kernel(data1, data2):
    full, _ = _run(data1, data2, trace=False)
    return full


# revision 3
# speedup vs baseline: 5.1206x; 5.1206x over previous
"""Trainium2 Bass kernel: 8-NN retrieval with inverse-distance weighting.

Problem (full): data1 [4096, 1024] queries, data2 [8192, 1024] database.
  dist = pairwise Euclidean distance; top-8 nearest per query;
  w = 1/(dist+0.1); out = weighted average of the 8 neighbor vectors.

Sharding: data1 row-sharded across 8 NeuronCores (512 queries/core);
data2 replicated (the sharding_hint's data-parallel-over-queries option).

Per-core algorithm:
  score[n, m] = q_n . y_m - 0.5*||y_m||^2      (argmax score == argmin dist)
computed on the TensorE. Each fp32 input is split on the host into bf16
hi+lo halves; score = hi.hi + hi.lo + lo.hi runs at bf16 rate (3 cyc/row vs
fp32's 4) while keeping fp32-level accuracy: every bf16 product is exact in
fp32 and only the ~2^-18 lo.lo term is dropped (score error ~1e-4 vs the
top-8 boundary gaps ~1.6). The hi/lo operands are pre-tiled on the host so
each m-chunk load is one fully contiguous [128, 4096] block.

Top-8 per row comes from the DVE Max8/MaxIndex instructions (one pass each
over the [128, 8192] score tile). dist_k = sqrt(x2_n - 2*score_k) is
reconstructed for the 8 winners only; their database rows are fetched by
per-partition indirect DMA; the weighted average accumulates on the DVE.

Scheduling: chunk streams split across both HWDGE rings (sync + scalar);
the ||y||^2 input pass is interleaved into the first chunk loop so it never
front-runs chunk DMAs in a ring FIFO; PSUM evictions run on ACT; selection
is split into two passes so score buffers free early and the next group's
matmuls overlap the weight/gather tail.
"""

import sys

sys.path.insert(0, "/opt/trn_rl_repo")

import numpy as np

P = 128
D = 1024
M = 8192
NQ = 512          # queries per core
KD = D // P       # 8 contraction tiles
MC = 512          # m-chunk width (one PSUM bank)
NMC = M // MC     # 16
NT = NQ // P      # 4 query tiles per core
MT = M // P       # 64 database row-tiles
K = 8
CONST = 0.1
EPS = 1e-12
N_CORES = 8

_CACHE = {}


def _build_nc():
    import concourse.bacc as bacc
    import concourse.bass as bass
    import concourse.mybir as mybir
    from concourse.masks import make_identity
    from concourse.tile import TileContext

    f32 = mybir.dt.float32
    u32 = mybir.dt.uint32
    bf = mybir.dt.bfloat16
    AF = mybir.ActivationFunctionType
    OP = mybir.AluOpType

    n_tile_groups = ((0, 2), (2, 4))

    nc = bacc.Bacc()

    # host-tiled: qt_*[p, kd*NQ+n] = q[n, kd*128+p]
    #             dbt_*[mc, p, kd*MC+j] = y[mc*512+j, kd*128+p]
    qt_hi = nc.dram_tensor("qt_hi", [P, KD * NQ], bf, kind="ExternalInput")
    qt_lo = nc.dram_tensor("qt_lo", [P, KD * NQ], bf, kind="ExternalInput")
    dbt_hi = nc.dram_tensor("dbt_hi", [NMC, P, KD * MC], bf, kind="ExternalInput")
    dbt_lo = nc.dram_tensor("dbt_lo", [NMC, P, KD * MC], bf, kind="ExternalInput")
    qn = nc.dram_tensor("qn", [NQ, D], f32, kind="ExternalInput")
    dbn = nc.dram_tensor("dbn", [M, D], f32, kind="ExternalInput")
    out = nc.dram_tensor("out", [NQ, D], f32, kind="ExternalOutput")

    with TileContext(nc) as tc:
        with (
            tc.tile_pool(name="persist", bufs=1) as pp,
            tc.tile_pool(name="stream", bufs=2) as sp,
            tc.tile_pool(name="scorep", bufs=1) as scp,
            tc.tile_pool(name="neighp", bufs=1) as nbp,
            tc.tile_pool(name="psum", bufs=6, space="PSUM") as psp,
            tc.tile_pool(name="psum1", bufs=1, space="PSUM") as ps1,
        ):
            first_group = n_tile_groups[0]

            # ---- identity (for the y2 transpose) ----
            ident = pp.tile([P, P], f32)
            make_identity(nc, ident[:])

            # ---- queries, d-major (matmul lhsT), columns kd*NQ + n ----
            qt_sb_hi = pp.tile([P, KD * NQ], bf)
            nc.sync.dma_start(out=qt_sb_hi[:], in_=qt_hi[:, :])
            qt_sb_lo = pp.tile([P, KD * NQ], bf)
            nc.scalar.dma_start(out=qt_sb_lo[:], in_=qt_lo[:, :])

            x2col = pp.tile([P, NT], f32)
            junk = pp.tile([P, D], f32)  # ACT mandatory full output, discarded
            y2blk = pp.tile([P, MT], f32)
            y2rep = pp.tile([P, M], f32)

            def emit_qn_load(t):
                qn_t = sp.tile([P, D], f32, tag="ldn", name="qn_t")
                nc.scalar.dma_start(out=qn_t[:], in_=qn[t * P : (t + 1) * P, :])
                nc.scalar.activation(
                    out=junk[:], in_=qn_t[:], func=AF.Square,
                    accum_out=x2col[:, t : t + 1],
                )

            def emit_dbn_load(t, eng):
                dbn_t = sp.tile([P, D], f32, tag="ldn", name="dbn_t")
                eng.dma_start(out=dbn_t[:], in_=dbn[t * P : (t + 1) * P, :])
                nc.scalar.activation(
                    out=junk[:], in_=dbn_t[:], func=AF.Square,
                    accum_out=y2blk[:, t : t + 1],
                )

            def emit_y2_finish():
                # transpose then scale by -0.5: y2t[t, p] = -0.5*y2[t*128+p]
                y2t_ps = ps1.tile([MT, P], f32)
                nc.tensor.transpose(out=y2t_ps[:], in_=y2blk[:], identity=ident[:])
                y2t = pp.tile([MT, P], f32)
                nc.vector.tensor_scalar_mul(y2t[:], y2t_ps[:], -0.5)
                # flatten into partition row 0 of y2rep (SWDGE), then doubling
                nc.gpsimd.dma_start(
                    out=y2rep[0:1, :].rearrange("o (t j) -> o t j", t=MT),
                    in_=y2t[:, :],
                )
                s = 1
                while s < P:
                    nc.gpsimd.dma_start(out=y2rep[s : 2 * s, :], in_=y2rep[0:s, :])
                    s *= 2

            # ---- selection scratch ----
            top8 = pp.tile([P, NT * K], f32)
            idx8 = pp.tile([P, NT * K], u32)
            wts = pp.tile([P, NT * K], f32)
            dsc = pp.tile([P, NT * K], f32)
            wsum = pp.tile([P, NT], f32)
            winv = pp.tile([P, NT], f32)

            scores = [
                scp.tile([P, M], f32, tag=f"score{i}", name=f"score{i}")
                for i in range(2)
            ]

            for (t0, t1) in n_tile_groups:
                # ---- score matmuls over all m-chunks ----
                for mc in range(NMC):
                    dbt_c_hi = sp.tile([P, KD * MC], bf, tag="dbt_c_hi",
                                       name="dbt_c_hi")
                    nc.sync.dma_start(out=dbt_c_hi[:], in_=dbt_hi[mc])
                    dbt_c_lo = sp.tile([P, KD * MC], bf, tag="dbt_c_lo",
                                       name="dbt_c_lo")
                    nc.scalar.dma_start(out=dbt_c_lo[:], in_=dbt_lo[mc])
                    terms = [
                        (qt_sb_hi, dbt_c_hi),
                        (qt_sb_hi, dbt_c_lo),
                        (qt_sb_lo, dbt_c_hi),
                    ]

                    # interleave the x2/y2 input loads into the first group's
                    # chunk loop so they never front-run chunk DMAs in a ring
                    # FIFO.
                    if (t0, t1) == first_group:
                        if mc == 0:
                            for t in range(NT):
                                emit_qn_load(t)
                        for i in range(4):
                            tt = mc * 4 + i
                            if tt < MT:
                                emit_dbn_load(
                                    tt, nc.sync if i % 2 == 0 else nc.scalar
                                )
                        if mc == NMC - 1:
                            emit_y2_finish()

                    for t in range(t0, t1):
                        ps = psp.tile([P, MC], f32, tag="mmps", name="mmps")
                        nmm = len(terms) * KD
                        i = 0
                        for lhs_sb, rhs_sb in terms:
                            for d in range(KD):
                                nc.tensor.matmul(
                                    ps[:],
                                    lhsT=lhs_sb[:, d * NQ + t * P : d * NQ + t * P + P],
                                    rhs=rhs_sb[:, d * MC : (d + 1) * MC],
                                    start=(i == 0),
                                    stop=(i == nmm - 1),
                                )
                                i += 1
                        # evict on ACT: plain copy (y2 folded in bulk later)
                        nc.scalar.activation(
                            out=scores[t - t0][:, mc * MC : (mc + 1) * MC],
                            in_=ps[:],
                            func=AF.Copy,
                        )

                # ---- pass A: fold y2 + top-8 + indices + launch gathers ----
                # (frees the score tiles as early as possible so the next
                # group's evictions can proceed while weights/gathers run)
                nbs = {}
                for t in range(t0, t1):
                    sc = scores[t - t0]
                    nc.vector.tensor_tensor(
                        out=sc[:], in0=sc[:], in1=y2rep[:], op=OP.add
                    )
                    sl = slice(t * K, (t + 1) * K)
                    nc.vector.max(out=top8[:, sl], in_=sc[:])
                    nc.vector.max_index(
                        out=idx8[:, sl], in_max=top8[:, sl], in_values=sc[:]
                    )
                    # half-size neighbor buffer; 8 rows gathered in 2 rounds
                    nb = nbp.tile([P, (K // 2) * D], f32, tag=f"nb{t % 2}",
                                  name="nb")
                    nbs[t] = nb
                    for k in range(K // 2):
                        nc.gpsimd.indirect_dma_start(
                            out=nb[:, k * D : (k + 1) * D],
                            out_offset=None,
                            in_=dbn[:, :],
                            in_offset=bass.IndirectOffsetOnAxis(
                                ap=idx8[:, t * K + k : t * K + k + 1], axis=0
                            ),
                        )

                # ---- pass B: weights + weighted average + store ----
                for t in range(t0, t1):
                    sl = slice(t * K, (t + 1) * K)
                    nb = nbs[t]
                    # d2 = x2 - 2*score  (>= 0 up to rounding)
                    nc.vector.scalar_tensor_tensor(
                        out=dsc[:, sl],
                        in0=top8[:, sl],
                        scalar=-2.0,
                        in1=x2col[:, t : t + 1].to_broadcast([P, K]),
                        op0=OP.mult,
                        op1=OP.add,
                    )
                    nc.vector.tensor_scalar_max(dsc[:, sl], dsc[:, sl], EPS)
                    nc.scalar.activation(out=dsc[:, sl], in_=dsc[:, sl], func=AF.Sqrt)
                    nc.vector.tensor_scalar_add(dsc[:, sl], dsc[:, sl], CONST)
                    nc.vector.reciprocal(out=wts[:, sl], in_=dsc[:, sl])
                    nc.vector.reduce_sum(
                        out=wsum[:, t : t + 1], in_=wts[:, sl],
                        axis=mybir.AxisListType.X,
                    )
                    nc.vector.reciprocal(out=winv[:, t : t + 1], in_=wsum[:, t : t + 1])
                    acc = sp.tile([P, D], f32, tag="acc", name="acc")
                    H = K // 2
                    for k in range(H):
                        nbk = nb[:, k * D : (k + 1) * D]
                        if k == 0:
                            nc.vector.tensor_scalar_mul(
                                acc[:], nbk, wts[:, t * K : t * K + 1]
                            )
                        else:
                            nc.vector.scalar_tensor_tensor(
                                out=acc[:],
                                in0=nbk,
                                scalar=wts[:, t * K + k : t * K + k + 1],
                                in1=acc[:],
                                op0=OP.mult,
                                op1=OP.add,
                            )
                    # second round of gathers reuses the buffer slots
                    for k in range(H, K):
                        nc.gpsimd.indirect_dma_start(
                            out=nb[:, (k - H) * D : (k - H + 1) * D],
                            out_offset=None,
                            in_=dbn[:, :],
                            in_offset=bass.IndirectOffsetOnAxis(
                                ap=idx8[:, t * K + k : t * K + k + 1], axis=0
                            ),
                        )
                    for k in range(H, K):
                        nc.vector.scalar_tensor_tensor(
                            out=acc[:],
                            in0=nb[:, (k - H) * D : (k - H + 1) * D],
                            scalar=wts[:, t * K + k : t * K + k + 1],
                            in1=acc[:],
                            op0=OP.mult,
                            op1=OP.add,
                        )
                    nc.vector.tensor_scalar_mul(acc[:], acc[:], winv[:, t : t + 1])
                    nc.sync.dma_start(out=out[t * P : (t + 1) * P, :], in_=acc[:])

    nc.finalize()
    return nc


def _shard_inputs(data1, data2):
    import ml_dtypes

    bf16 = ml_dtypes.bfloat16
    data1 = np.ascontiguousarray(np.asarray(data1, dtype=np.float32))
    data2 = np.ascontiguousarray(np.asarray(data2, dtype=np.float32))

    def tile_db(x):  # [1024(d), 8192(m)] -> [NMC, P, KD*MC]
        return np.ascontiguousarray(
            x.reshape(KD, P, NMC, MC).transpose(2, 1, 0, 3).reshape(NMC, P, KD * MC)
        )

    def tile_q(x):  # [1024(d), NQ] -> [P, KD*NQ]
        return np.ascontiguousarray(
            x.reshape(KD, P, NQ).transpose(1, 0, 2).reshape(P, KD * NQ)
        )

    db_hi32 = data2.T.astype(bf16).astype(np.float32)
    dbt_hi = tile_db(db_hi32.astype(bf16))
    dbt_lo = tile_db((data2.T - db_hi32).astype(bf16))
    in_maps = []
    for c in range(N_CORES):
        q = data1[c * NQ : (c + 1) * NQ]
        q_hi32 = q.T.astype(bf16).astype(np.float32)
        in_maps.append(
            {
                "qt_hi": tile_q(q_hi32.astype(bf16)),
                "qt_lo": tile_q((q.T - q_hi32).astype(bf16)),
                "dbt_hi": dbt_hi,
                "dbt_lo": dbt_lo,
                "qn": np.ascontiguousarray(q),
                "dbn": data2,
            }
        )
    return in_maps


def _run(data1, data2, trace=False, trace_kwargs=None):
    from concourse.bass_utils import run_bass_kernel_spmd

    nc = _CACHE.get("nc")
    if nc is None:
        nc = _build_nc()
        _CACHE["nc"] = nc
    in_maps = _shard_inputs(data1, data2)
    res = run_bass_kernel_spmd(
        nc,
        in_maps,
        core_ids=list(range(N_CORES)),
        trace=trace,
        trace_kwargs=trace_kwargs or {},
    )
    full = np.concatenate([res.results[c]["out"] for c in range(N_CORES)], axis=0)
    return full, res


def kernel(data1, data2):
    full, _ = _run(data1, data2, trace=False)
    return full
